# revision 19
# baseline (speedup 1.0000x reference)
"""Trainium2 Bass kernel for nn_CamMemory (soft cross-entropy vs. memory bank).

Computes: x = normalize(inputs); logits = x @ features.T / TEMP;
loss = mean_b( lse(logits_b) - dot(softmax(targets_b), logits_b) )

Sharding: features/targets split row-wise (N dim) across 8 cores; inputs
replicated.  Each core returns partial stats (s, u, p) per batch row:
  s = sum_n exp(logits - SHIFT)      (partial sum-exp, fixed shift; |logits|<=21)
  u = sum_n exp(targets - 1)         (partial softmax denominator; targets in [0,1))
  p = sum_n exp(targets - 1)*logits  (partial weighted logit sum)
Host combines: loss = mean_b( SHIFT + log(sum s) - (sum p)/(sum u) ).

Per-core pipeline (DMA budget is the 16.8MB feature load; everything else
stays off the DMA subsystem):
  - SWDGE cast-DMA features f32 DRAM -> bf16 SBUF, natural layout (n on
    partitions), 1MB chunks.
  - PE transpose-mode matmuls flip each 128x128 block into PSUM staging
    banks (8 blocks per bank), putting D on partitions.
  - Batched PSUM->SBUF copies (DVE/ACT alternating) build featT chunks.
  - bf16 matmuls with xT (DMA-xbar transposed, 1/TEMP and 1/||x|| folded in)
    stationary accumulate logits [64, 128] per chunk.
  - Fused exp+row-sum on ACT; mul+row-sum on DVE.
"""

import numpy as np

import concourse.bacc as bacc
import concourse.mybir as mybir
import concourse.tile as tile
from concourse.masks import make_identity
from concourse.tile_rust import add_dep_helper

B = 64
D = 2048
N = 16384
NUM_CORES = 8
NSH = N // NUM_CORES  # 2048 rows of features per core
TEMP = 0.05
SHIFT = 21.0  # |logits| <= (1/TEMP)*|x.f| <= 20*(1+eps) since both unit-norm

F32 = mybir.dt.float32
BF16 = mybir.dt.bfloat16


def build_nc(d=D, nsh=NSH, b=B, debug=False):
    """Build the single-core Bass program (SPMD: same program, 8 shards)."""
    kc = d // 128     # contraction chunks (d on partitions)
    nch = nsh // 128  # feature-row chunks
    TG = min(8, kc)   # transposed blocks staged per PSUM bank
    ngrp = kc // TG

    nc = bacc.Bacc("TRN2", target_bir_lowering=False, debug=debug)

    inputs_d = nc.dram_tensor("inputs", [b, d], F32, kind="ExternalInput")
    targets_d = nc.dram_tensor("targets", [b, nsh], F32, kind="ExternalInput")
    features_d = nc.dram_tensor("features", [nsh, d], F32, kind="ExternalInput")
    out_d = nc.dram_tensor("out", [b, 4], F32, kind="ExternalOutput")

    with tile.TileContext(nc) as tc:
        with (
            tc.tile_pool(name="small", bufs=1) as small,
            tc.tile_pool(name="nat", bufs=5) as natp,
            tc.tile_pool(name="ft", bufs=4) as ftp,
            tc.tile_pool(name="epi", bufs=4) as epi,
            tc.tile_pool(name="tps", bufs=4, space="PSUM") as tpsp,
            tc.tile_pool(name="psum", bufs=4, space="PSUM") as psp,
        ):
            # constants
            ident = small.tile([128, 128], BF16)
            make_identity(nc, ident[:])
            bias_m1 = small.tile([b, 1], F32)
            nc.vector.memset(bias_m1[:], -1.0)
            bias_shift = small.tile([b, 1], F32)
            nc.vector.memset(bias_shift[:], -float(SHIFT))

            # ---- x preparation: x = (inputs/||inputs||) / TEMP, bf16, transposed
            xin = small.tile([b, d], F32)
            nc.sync.dma_start(xin[:], inputs_d[:])
            sq = small.tile([b, d], F32)
            ss = small.tile([b, 1], F32)
            nc.scalar.activation(
                sq[:], xin[:], mybir.ActivationFunctionType.Square,
                accum_out=ss[:],
            )
            # inv = (1/TEMP)/sqrt(ss):  sqrt(ss*TEMP^2) then reciprocal
            srt = small.tile([b, 1], F32)
            i_sqrt = nc.scalar.activation(
                srt[:], ss[:], mybir.ActivationFunctionType.Sqrt,
                scale=float(TEMP) * float(TEMP),
            )
            inv = small.tile([b, 1], F32)
            nc.vector.reciprocal(inv[:], srt[:])
            # x padded to 128 partitions so its PE transposes exactly mirror
            # the feature-block pattern (a DMA-xbar transpose here would
            # force a full DMA-pipeline flush against the streaming casts)
            xbp = small.tile([128, d], BF16)
            nc.gpsimd.memset(xbp[b:128, :], 0.0)
            i_tsmul = nc.vector.tensor_scalar_mul(xbp[:b, :], xin[:], inv[:])
            xT = small.tile([128, kc, 128], BF16)

            # ---- targets: exp(t - 1) and its row-sum u
            tg = small.tile([b, nsh], F32)
            nc.sync.dma_start(tg[:], targets_d[:])
            et = small.tile([b, nsh], F32)
            u = small.tile([b, 1], F32)
            i_etexp = nc.scalar.activation(
                et[:], tg[:], mybir.ActivationFunctionType.Exp,
                bias=bias_m1[:], accum_out=u[:],
            )
            # et-exp must not preempt the x-chain on ACT
            add_dep_helper(i_etexp.ins, i_sqrt.ins, sync=False,
                           reason="x-chain first on ACT")

            # x transposes through the same PSUM staging pool as features
            for g in range(ngrp):
                tp = tpsp.tile([128, TG, 128], BF16)
                for j in range(TG):
                    k = g * TG + j
                    nc.tensor.transpose(
                        tp[:, j, :], xbp[:, k * 128:(k + 1) * 128], ident[:])
                i_xcp = nc.vector.tensor_copy(xT[:, g * TG:(g + 1) * TG, :], tp[:])
                add_dep_helper(i_xcp.ins, i_tsmul.ins, sync=False,
                               reason="x-chain first on DVE")

            # ---- features pipeline: per 128-row chunk
            s_parts = small.tile([b, nch], F32)
            p_parts = small.tile([b, nch], F32)
            for c in range(nch):
                natc = natp.tile([128, d], BF16)
                # SWDGE cast-DMA: f32 DRAM -> bf16 SBUF (the only big DMA)
                nc.gpsimd.dma_start(natc[:], features_d[c * 128:(c + 1) * 128, :])

                # PE transposes 128x128 blocks into PSUM staging; batched
                # copies move them to SBUF as featT [128(d), kc, 128(n)].
                ftc = ftp.tile([128, kc, 128], BF16)
                for g in range(ngrp):
                    tp = tpsp.tile([128, TG, 128], BF16)
                    for j in range(TG):
                        k = g * TG + j
                        nc.tensor.transpose(
                            tp[:, j, :], natc[:, k * 128:(k + 1) * 128], ident[:])
                    dst = ftc[:, g * TG:(g + 1) * TG, :]
                    if (c * ngrp + g) * 8 % 13 < 8:
                        i_cp = nc.vector.tensor_copy(dst, tp[:])
                        if c < 4:
                            # copies must not preempt the x-chain on DVE
                            add_dep_helper(i_cp.ins, i_tsmul.ins, sync=False,
                                           reason="x-chain first on DVE")
                    else:
                        i_cp = nc.scalar.copy(dst, tp[:])
                        if c < 4:
                            add_dep_helper(i_cp.ins, i_sqrt.ins, sync=False,
                                           reason="x-chain first on ACT")

                ps = psp.tile([b, 128], F32)
                for k in range(kc):
                    nc.tensor.matmul(
                        ps[:], xT[:, k, 0:b], ftc[:, k, :],
                        start=(k == 0), stop=(k == kc - 1),
                    )
                # s_part[c] = sum_n exp(logits - SHIFT)   (fused on ACT)
                el = epi.tile([b, 128], F32)
                nc.scalar.activation(
                    el[:], ps[:], mybir.ActivationFunctionType.Exp,
                    bias=bias_shift[:], accum_out=s_parts[:, c:c + 1],
                )
                # p_part[c] = sum_n exp_t * logits        (DVE mul + reduce)
                pm = epi.tile([b, 128], F32)
                nc.vector.tensor_mul(pm[:], et[:, c * 128:(c + 1) * 128], ps[:])
                nc.vector.reduce_sum(
                    p_parts[:, c:c + 1], pm[:], axis=mybir.AxisListType.X)

            # ---- final per-core reduction and output
            sbout = small.tile([b, 4], F32)
            nc.vector.reduce_sum(
                sbout[:, 0:1], s_parts[:], axis=mybir.AxisListType.X)
            nc.vector.tensor_copy(sbout[:, 1:2], u[:])
            nc.vector.reduce_sum(
                sbout[:, 2:3], p_parts[:], axis=mybir.AxisListType.X)
            nc.vector.memset(sbout[:, 3:4], 0.0)
            nc.sync.dma_start(out_d[:], sbout[:])

    nc.compile()
    return nc


_NC_CACHE = None


def _run(inputs, trace=False, **spmd_kwargs):
    global _NC_CACHE
    from concourse.bass_utils import run_bass_kernel_spmd

    x = np.ascontiguousarray(np.asarray(inputs["inputs"], dtype=np.float32))
    t = np.asarray(inputs["targets"], dtype=np.float32)
    f = np.asarray(inputs["features"], dtype=np.float32)
    # cid is unused by the reference computation.

    if _NC_CACHE is None:
        _NC_CACHE = build_nc(debug=False)
    nc = _NC_CACHE

    in_maps = []
    for c in range(NUM_CORES):
        in_maps.append({
            "inputs": x,
            "targets": np.ascontiguousarray(t[:, c * NSH:(c + 1) * NSH]),
            "features": np.ascontiguousarray(f[c * NSH:(c + 1) * NSH, :]),
        })

    res = run_bass_kernel_spmd(
        nc, in_maps, core_ids=list(range(NUM_CORES)), trace=trace, **spmd_kwargs)
    outs = np.stack([r["out"] for r in res.results])  # [8, B, 4]

    outs64 = outs.astype(np.float64)
    s = outs64[:, :, 0].sum(0)
    u = outs64[:, :, 1].sum(0)
    p = outs64[:, :, 2].sum(0)
    lse = SHIFT + np.log(s)
    loss = np.mean(lse - p / u)
    return np.float32(loss), res


def kernel(**inputs: np.ndarray) -> np.ndarray:
    loss, _ = _run(inputs)
    return loss


# revision 20
# speedup vs baseline: 1.1274x; 1.1274x over previous
"""Trainium2 Bass kernel for nn_CamMemory (soft cross-entropy vs. memory bank).

Computes: x = normalize(inputs); logits = x @ features.T / TEMP;
loss = mean_b( lse(logits_b) - dot(softmax(targets_b), logits_b) )

Sharding: features/targets split row-wise (N dim) across 8 cores; inputs
replicated.  Each core returns partial stats (s, u, p) per batch row:
  s = sum_n exp(logits - SHIFT)      (partial sum-exp, fixed shift; |logits|<=21)
  u = sum_n exp(targets - 1)         (partial softmax denominator; targets in [0,1))
  p = sum_n exp(targets - 1)*logits  (partial weighted logit sum)
Host combines: loss = mean_b( SHIFT + log(sum s) - (sum p)/(sum u) ).

Per-core pipeline (DMA budget is the 16.8MB feature load; everything else
stays off the DMA subsystem):
  - SWDGE cast-DMA features f32 DRAM -> bf16 SBUF, natural layout (n on
    partitions), 1MB chunks.
  - PE transpose-mode matmuls flip each 128x128 block into PSUM staging
    banks (8 blocks per bank), putting D on partitions.
  - Batched PSUM->SBUF copies (DVE/ACT alternating) build featT chunks.
  - bf16 matmuls with xT (DMA-xbar transposed, 1/TEMP and 1/||x|| folded in)
    stationary accumulate logits [64, 128] per chunk.
  - Fused exp+row-sum on ACT; mul+row-sum on DVE.
"""

import numpy as np

import concourse.bacc as bacc
import concourse.mybir as mybir
import concourse.tile as tile
from concourse.masks import make_identity
from concourse.tile_rust import add_dep_helper

B = 64
D = 2048
N = 16384
NUM_CORES = 8
NSH = N // NUM_CORES  # 2048 rows of features per core
TEMP = 0.05
SHIFT = 21.0  # |logits| <= (1/TEMP)*|x.f| <= 20*(1+eps) since both unit-norm

F32 = mybir.dt.float32
BF16 = mybir.dt.bfloat16


def build_nc(d=D, nsh=NSH, b=B, debug=False):
    """Build the single-core Bass program (SPMD: same program, 8 shards)."""
    kc = d // 128     # contraction chunks (d on partitions)
    nch = nsh // 128  # feature-row chunks
    TG = min(8, kc)   # transposed blocks staged per PSUM bank
    ngrp = kc // TG

    nc = bacc.Bacc("TRN2", target_bir_lowering=False, debug=debug)

    inputs_d = nc.dram_tensor("inputs", [b, d], F32, kind="ExternalInput")
    targets_d = nc.dram_tensor("targets", [b, nsh], F32, kind="ExternalInput")
    features_d = nc.dram_tensor("features", [nsh, d], F32, kind="ExternalInput")
    out_d = nc.dram_tensor("out", [b, 4], F32, kind="ExternalOutput")

    with tile.TileContext(nc) as tc:
        with (
            tc.tile_pool(name="small", bufs=1) as small,
            tc.tile_pool(name="nat", bufs=5) as natp,
            tc.tile_pool(name="ft", bufs=4) as ftp,
            tc.tile_pool(name="epi", bufs=4) as epi,
            tc.tile_pool(name="tps", bufs=4, space="PSUM") as tpsp,
            tc.tile_pool(name="psum", bufs=4, space="PSUM") as psp,
        ):
            # constants
            ident = small.tile([128, 128], BF16)
            make_identity(nc, ident[:])
            bias_m1 = small.tile([b, 1], F32)
            nc.vector.memset(bias_m1[:], -1.0)
            bias_shift = small.tile([b, 1], F32)
            nc.vector.memset(bias_shift[:], -float(SHIFT))

            # ---- x preparation: x = (inputs/||inputs||) / TEMP, bf16, transposed
            xin = small.tile([b, d], F32)
            nc.sync.dma_start(xin[:], inputs_d[:])
            sq = small.tile([b, d], F32)
            ss = small.tile([b, 1], F32)
            nc.scalar.activation(
                sq[:], xin[:], mybir.ActivationFunctionType.Square,
                accum_out=ss[:],
            )
            # inv = (1/TEMP)/sqrt(ss):  sqrt(ss*TEMP^2) then reciprocal
            srt = small.tile([b, 1], F32)
            i_sqrt = nc.scalar.activation(
                srt[:], ss[:], mybir.ActivationFunctionType.Sqrt,
                scale=float(TEMP) * float(TEMP),
            )
            inv = small.tile([b, 1], F32)
            nc.vector.reciprocal(inv[:], srt[:])
            # x padded to 128 partitions so its PE transposes exactly mirror
            # the feature-block pattern (a DMA-xbar transpose here would
            # force a full DMA-pipeline flush against the streaming casts)
            xbp = small.tile([128, d], BF16)
            nc.gpsimd.memset(xbp[b:128, :], 0.0)
            i_tsmul = nc.vector.tensor_scalar_mul(xbp[:b, :], xin[:], inv[:])
            xT = small.tile([128, kc, 128], BF16)

            # ---- targets: exp(t - 1) and its row-sum u
            tg = small.tile([b, nsh], F32)
            nc.sync.dma_start(tg[:], targets_d[:])
            et = small.tile([b, nsh], F32)
            u = small.tile([b, 1], F32)
            i_etexp = nc.scalar.activation(
                et[:], tg[:], mybir.ActivationFunctionType.Exp,
                bias=bias_m1[:], accum_out=u[:],
            )
            # et-exp must not preempt the x-chain on ACT
            add_dep_helper(i_etexp.ins, i_sqrt.ins, sync=False,
                           reason="x-chain first on ACT")

            # x transposes through the same PSUM staging pool as features
            for g in range(ngrp):
                tp = tpsp.tile([128, TG, 128], BF16)
                for j in range(TG):
                    k = g * TG + j
                    nc.tensor.transpose(
                        tp[:, j, :], xbp[:, k * 128:(k + 1) * 128], ident[:])
                i_xcp = nc.vector.tensor_copy(xT[:, g * TG:(g + 1) * TG, :], tp[:])
                add_dep_helper(i_xcp.ins, i_tsmul.ins, sync=False,
                               reason="x-chain first on DVE")

            # ---- features pipeline: per 128-row chunk, software-pipelined
            # by one chunk so the logits matmuls of chunk c-1 run while the
            # PSUM->SBUF copies of chunk c are still in flight (the PE never
            # sits waiting on a copy it just enabled).
            s_parts = small.tile([b, nch], F32)
            p_parts = small.tile([b, nch], F32)

            def mms_and_epi(c, ftc):
                ps = psp.tile([b, 128], F32)
                for k in range(kc):
                    nc.tensor.matmul(
                        ps[:], xT[:, k, 0:b], ftc[:, k, :],
                        start=(k == 0), stop=(k == kc - 1),
                    )
                # s_part[c] = sum_n exp(logits - SHIFT)   (fused on ACT)
                el = epi.tile([b, 128], F32)
                nc.scalar.activation(
                    el[:], ps[:], mybir.ActivationFunctionType.Exp,
                    bias=bias_shift[:], accum_out=s_parts[:, c:c + 1],
                )
                # p_part[c] = sum_n exp_t * logits        (DVE mul + reduce)
                pm = epi.tile([b, 128], F32)
                nc.vector.tensor_mul(pm[:], et[:, c * 128:(c + 1) * 128], ps[:])
                nc.vector.reduce_sum(
                    p_parts[:, c:c + 1], pm[:], axis=mybir.AxisListType.X)

            prev = None
            for c in range(nch):
                natc = natp.tile([128, d], BF16)
                # SWDGE cast-DMA: f32 DRAM -> bf16 SBUF (the only big DMA)
                nc.gpsimd.dma_start(natc[:], features_d[c * 128:(c + 1) * 128, :])

                # PE transposes 128x128 blocks into PSUM staging; batched
                # copies move them to SBUF as featT [128(d), kc, 128(n)].
                ftc = ftp.tile([128, kc, 128], BF16)
                for g in range(ngrp):
                    tp = tpsp.tile([128, TG, 128], BF16)
                    for j in range(TG):
                        k = g * TG + j
                        nc.tensor.transpose(
                            tp[:, j, :], natc[:, k * 128:(k + 1) * 128], ident[:])
                    dst = ftc[:, g * TG:(g + 1) * TG, :]
                    if (c * ngrp + g) * 8 % 13 < 8:
                        i_cp = nc.vector.tensor_copy(dst, tp[:])
                        if c < 4:
                            # copies must not preempt the x-chain on DVE
                            add_dep_helper(i_cp.ins, i_tsmul.ins, sync=False,
                                           reason="x-chain first on DVE")
                    else:
                        i_cp = nc.scalar.copy(dst, tp[:])
                        if c < 4:
                            add_dep_helper(i_cp.ins, i_sqrt.ins, sync=False,
                                           reason="x-chain first on ACT")

                if prev is not None:
                    mms_and_epi(prev[0], prev[1])
                prev = (c, ftc)
            mms_and_epi(prev[0], prev[1])

            # ---- final per-core reduction and output
            sbout = small.tile([b, 4], F32)
            nc.vector.reduce_sum(
                sbout[:, 0:1], s_parts[:], axis=mybir.AxisListType.X)
            nc.vector.tensor_copy(sbout[:, 1:2], u[:])
            nc.vector.reduce_sum(
                sbout[:, 2:3], p_parts[:], axis=mybir.AxisListType.X)
            nc.vector.memset(sbout[:, 3:4], 0.0)
            nc.sync.dma_start(out_d[:], sbout[:])

    nc.compile()
    return nc


_NC_CACHE = None


def _run(inputs, trace=False, **spmd_kwargs):
    global _NC_CACHE
    from concourse.bass_utils import run_bass_kernel_spmd

    x = np.ascontiguousarray(np.asarray(inputs["inputs"], dtype=np.float32))
    t = np.asarray(inputs["targets"], dtype=np.float32)
    f = np.asarray(inputs["features"], dtype=np.float32)
    # cid is unused by the reference computation.

    if _NC_CACHE is None:
        _NC_CACHE = build_nc(debug=False)
    nc = _NC_CACHE

    in_maps = []
    for c in range(NUM_CORES):
        in_maps.append({
            "inputs": x,
            "targets": np.ascontiguousarray(t[:, c * NSH:(c + 1) * NSH]),
            "features": np.ascontiguousarray(f[c * NSH:(c + 1) * NSH, :]),
        })

    res = run_bass_kernel_spmd(
        nc, in_maps, core_ids=list(range(NUM_CORES)), trace=trace, **spmd_kwargs)
    outs = np.stack([r["out"] for r in res.results])  # [8, B, 4]

    outs64 = outs.astype(np.float64)
    s = outs64[:, :, 0].sum(0)
    u = outs64[:, :, 1].sum(0)
    p = outs64[:, :, 2].sum(0)
    lse = SHIFT + np.log(s)
    loss = np.mean(lse - p / u)
    return np.float32(loss), res


def kernel(**inputs: np.ndarray) -> np.ndarray:
    loss, _ = _run(inputs)
    return loss


# revision 22
# speedup vs baseline: 1.1341x; 1.0060x over previous
"""Trainium2 Bass kernel for nn_CamMemory (soft cross-entropy vs. memory bank).

Computes: x = normalize(inputs); logits = x @ features.T / TEMP;
loss = mean_b( lse(logits_b) - dot(softmax(targets_b), logits_b) )

Sharding: features/targets split row-wise (N dim) across 8 cores; inputs
replicated.  Each core returns partial stats (s, u, p) per batch row:
  s = sum_n exp(logits - SHIFT)      (partial sum-exp, fixed shift; |logits|<=21)
  u = sum_n exp(targets - 1)         (partial softmax denominator; targets in [0,1))
  p = sum_n exp(targets - 1)*logits  (partial weighted logit sum)
Host combines: loss = mean_b( SHIFT + log(sum s) - (sum p)/(sum u) ).

Per-core pipeline (DMA budget is the 16.8MB feature load; everything else
stays off the DMA subsystem):
  - SWDGE cast-DMA features f32 DRAM -> bf16 SBUF, natural layout (n on
    partitions), 1MB chunks.
  - PE transpose-mode matmuls flip each 128x128 block into PSUM staging
    banks (8 blocks per bank), putting D on partitions.
  - Batched PSUM->SBUF copies (DVE/ACT alternating) build featT chunks.
  - bf16 matmuls with xT (DMA-xbar transposed, 1/TEMP and 1/||x|| folded in)
    stationary accumulate logits [64, 128] per chunk.
  - Fused exp+row-sum on ACT; mul+row-sum on DVE.
"""

import numpy as np

import concourse.bacc as bacc
import concourse.mybir as mybir
import concourse.tile as tile
from concourse.masks import make_identity
from concourse.tile_rust import add_dep_helper

B = 64
D = 2048
N = 16384
NUM_CORES = 8
NSH = N // NUM_CORES  # 2048 rows of features per core
TEMP = 0.05
SHIFT = 21.0  # |logits| <= (1/TEMP)*|x.f| <= 20*(1+eps) since both unit-norm

F32 = mybir.dt.float32
BF16 = mybir.dt.bfloat16


def build_nc(d=D, nsh=NSH, b=B, debug=False):
    """Build the single-core Bass program (SPMD: same program, 8 shards)."""
    kc = d // 128     # contraction chunks (d on partitions)
    nch = nsh // 128  # feature-row chunks
    TG = min(8, kc)   # transposed blocks staged per PSUM bank
    ngrp = kc // TG

    nc = bacc.Bacc("TRN2", target_bir_lowering=False, debug=debug)

    inputs_d = nc.dram_tensor("inputs", [b, d], F32, kind="ExternalInput")
    targets_d = nc.dram_tensor("targets", [b, nsh], F32, kind="ExternalInput")
    features_d = nc.dram_tensor("features", [nsh, d], F32, kind="ExternalInput")
    out_d = nc.dram_tensor("out", [b, 4], F32, kind="ExternalOutput")

    with tile.TileContext(nc) as tc:
        with (
            tc.tile_pool(name="small", bufs=1) as small,
            tc.tile_pool(name="nat", bufs=5) as natp,
            tc.tile_pool(name="ft", bufs=4) as ftp,
            tc.tile_pool(name="epi", bufs=4) as epi,
            tc.tile_pool(name="tps", bufs=4, space="PSUM") as tpsp,
            tc.tile_pool(name="psum", bufs=4, space="PSUM") as psp,
        ):
            # constants
            ident = small.tile([128, 128], BF16)
            make_identity(nc, ident[:])
            bias_m1 = small.tile([b, 1], F32)
            nc.vector.memset(bias_m1[:], -1.0)
            bias_shift = small.tile([b, 1], F32)
            nc.vector.memset(bias_shift[:], -float(SHIFT))

            # ---- x preparation: x = (inputs/||inputs||) / TEMP, bf16, transposed
            xin = small.tile([b, d], F32)
            nc.sync.dma_start(xin[:], inputs_d[:])
            sq = small.tile([b, d], F32)
            ss = small.tile([b, 1], F32)
            nc.scalar.activation(
                sq[:], xin[:], mybir.ActivationFunctionType.Square,
                accum_out=ss[:],
            )
            # inv = (1/TEMP)/sqrt(ss):  sqrt(ss*TEMP^2) then reciprocal
            srt = small.tile([b, 1], F32)
            i_sqrt = nc.scalar.activation(
                srt[:], ss[:], mybir.ActivationFunctionType.Sqrt,
                scale=float(TEMP) * float(TEMP),
            )
            inv = small.tile([b, 1], F32)
            nc.vector.reciprocal(inv[:], srt[:])
            # x padded to 128 partitions so its PE transposes exactly mirror
            # the feature-block pattern (a DMA-xbar transpose here would
            # force a full DMA-pipeline flush against the streaming casts)
            xbp = small.tile([128, d], BF16)
            nc.gpsimd.memset(xbp[b:128, :], 0.0)
            i_tsmul = nc.vector.tensor_scalar_mul(xbp[:b, :], xin[:], inv[:])
            xT = small.tile([128, kc, 128], BF16)

            # ---- targets: exp(t - 1) and its row-sum u
            tg = small.tile([b, nsh], F32)
            nc.sync.dma_start(tg[:], targets_d[:])
            et = small.tile([b, nsh], F32)
            u = small.tile([b, 1], F32)
            i_etexp = nc.scalar.activation(
                et[:], tg[:], mybir.ActivationFunctionType.Exp,
                bias=bias_m1[:], accum_out=u[:],
            )
            # et-exp must not preempt the x-chain on ACT
            add_dep_helper(i_etexp.ins, i_sqrt.ins, sync=False,
                           reason="x-chain first on ACT")

            # x transposes through the same PSUM staging pool as features
            for g in range(ngrp):
                tp = tpsp.tile([128, TG, 128], BF16)
                for j in range(TG):
                    k = g * TG + j
                    nc.tensor.transpose(
                        tp[:, j, :], xbp[:, k * 128:(k + 1) * 128], ident[:])
                i_xcp = nc.vector.tensor_copy(xT[:, g * TG:(g + 1) * TG, :], tp[:])
                add_dep_helper(i_xcp.ins, i_tsmul.ins, sync=False,
                               reason="x-chain first on DVE")

            # ---- features pipeline: per 128-row chunk, software-pipelined
            # by one chunk so the logits matmuls of chunk c-1 run while the
            # PSUM->SBUF copies of chunk c are still in flight (the PE never
            # sits waiting on a copy it just enabled).
            s_parts = small.tile([b, nch], F32)
            p_parts = small.tile([b, nch], F32)

            def mms_and_epi(c, ftc):
                ps = psp.tile([b, 128], F32)
                for k in range(kc):
                    nc.tensor.matmul(
                        ps[:], xT[:, k, 0:b], ftc[:, k, :],
                        start=(k == 0), stop=(k == kc - 1),
                    )
                # s_part[c] = sum_n exp(logits - SHIFT)   (fused on ACT)
                el = epi.tile([b, 128], F32)
                nc.scalar.activation(
                    el[:], ps[:], mybir.ActivationFunctionType.Exp,
                    bias=bias_shift[:], accum_out=s_parts[:, c:c + 1],
                )
                # p_part[c] = sum_n exp_t * logits        (DVE mul + reduce)
                pm = epi.tile([b, 128], F32)
                nc.vector.tensor_mul(pm[:], et[:, c * 128:(c + 1) * 128], ps[:])
                nc.vector.reduce_sum(
                    p_parts[:, c:c + 1], pm[:], axis=mybir.AxisListType.X)

            prev = None
            for c in range(nch):
                natc = natp.tile([128, d], BF16)
                # SWDGE cast-DMA: f32 DRAM -> bf16 SBUF (the only big DMA)
                nc.gpsimd.dma_start(natc[:], features_d[c * 128:(c + 1) * 128, :])

                # PE transposes 128x128 blocks into PSUM staging; batched
                # copies move them to SBUF as featT [128(d), kc, 128(n)].
                ftc = ftp.tile([128, kc, 128], BF16)
                for g in range(ngrp):
                    tp = tpsp.tile([128, TG, 128], BF16)
                    for j in range(TG):
                        k = g * TG + j
                        nc.tensor.transpose(
                            tp[:, j, :], natc[:, k * 128:(k + 1) * 128], ident[:])
                    dst = ftc[:, g * TG:(g + 1) * TG, :]
                    if (c * ngrp + g) * 8 % 13 < 8:
                        i_cp = nc.vector.tensor_copy(dst, tp[:])
                        if c < 4:
                            # copies must not preempt the x-chain on DVE
                            add_dep_helper(i_cp.ins, i_tsmul.ins, sync=False,
                                           reason="x-chain first on DVE")
                    else:
                        i_cp = nc.scalar.copy(dst, tp[:])
                        if c < 4:
                            add_dep_helper(i_cp.ins, i_sqrt.ins, sync=False,
                                           reason="x-chain first on ACT")

                if prev is not None:
                    mms_and_epi(prev[0], prev[1])
                prev = (c, ftc)
            mms_and_epi(prev[0], prev[1])

            # ---- final per-core reduction and output
            sbout = small.tile([b, 4], F32)
            nc.vector.reduce_sum(
                sbout[:, 0:1], s_parts[:], axis=mybir.AxisListType.X)
            nc.vector.tensor_copy(sbout[:, 1:2], u[:])
            nc.vector.reduce_sum(
                sbout[:, 2:3], p_parts[:], axis=mybir.AxisListType.X)
            nc.vector.memset(sbout[:, 3:4], 0.0)
            nc.sync.dma_start(out_d[:], sbout[:])

    nc.compile()
    return nc


_NC_CACHE = None


def _run(inputs, trace=False, **spmd_kwargs):
    global _NC_CACHE
    from concourse.bass_utils import run_bass_kernel_spmd

    x = np.ascontiguousarray(np.asarray(inputs["inputs"], dtype=np.float32))
    t = np.asarray(inputs["targets"], dtype=np.float32)
    f = np.asarray(inputs["features"], dtype=np.float32)
    # cid is unused by the reference computation.

    if _NC_CACHE is None:
        _NC_CACHE = build_nc(debug=False)
    nc = _NC_CACHE

    in_maps = []
    for c in range(NUM_CORES):
        in_maps.append({
            "inputs": x,
            "targets": np.ascontiguousarray(t[:, c * NSH:(c + 1) * NSH]),
            "features": np.ascontiguousarray(f[c * NSH:(c + 1) * NSH, :]),
        })

    res = run_bass_kernel_spmd(
        nc, in_maps, core_ids=list(range(NUM_CORES)), trace=trace, **spmd_kwargs)
    outs = np.stack([r["out"] for r in res.results])  # [8, B, 4]

    outs64 = outs.astype(np.float64)
    s = outs64[:, :, 0].sum(0)
    u = outs64[:, :, 1].sum(0)
    p = outs64[:, :, 2].sum(0)
    lse = SHIFT + np.log(s)
    loss = np.mean(lse - p / u)
    return np.float32(loss), res


def kernel(**inputs: np.ndarray) -> np.ndarray:
    loss, _ = _run(inputs)
    return loss
